# revision 1
# baseline (speedup 1.0000x reference)
"""Multi-head attention Bass kernel for Trainium2, 8-core SPMD.

Problem: B=2, S=2048, H=1024, 16 heads of 64 (torch-style MHA without
1/sqrt(d) scaling, key-padding mask, eval mode).

Sharding: core c handles batch b = c//4 and 4 heads (feature slice
256*(c%4) .. +256). Each core computes Q/K/V projections for its feature
slice over its batch, then attention for its 4 heads, producing
out[b, :, fslice]. Host concatenates.

Key-padding compaction: masked key positions contribute exactly
exp(-1e10) = 0 to softmax, so the host drops masked key/value rows and
pads to a multiple of 256 (typically 1280 of 2048 remain). Padding rows
get the -1e10 bias so they also contribute 0. Numerically identical to
the reference up to fp summation order.

Device-side layout: everything runs "transposed" (feature dim on
partitions); activations arrive pre-transposed from the host (layout
prep, like the weight transposes), so the device does no data
transposes except the tiny [65, q] output blocks:
  - Q^T, K^T [f, s]: scores S^T[kpos, q] = K^T.T @ Q^T (contraction
    d=64); the two heads of each 128-feature tile go to different PE
    row groups and run concurrently in the array
  - projections interleave with ACT-bound attention under a static
    8-bank PSUM plan; attention group g needs only K+Q[g], and V tiles
    are produced just ahead of their PV consumers
  - key-padding bias is per-kpos = per-partition -> folded into the
    exp() activation's bias operand
  - V kept [s, f] with an appended ones column per head, so the PV
    matmul gives out[0:64,:] = unnormalized out^T and out[64,:] = the
    softmax denominator
  - a small PE transpose of the [65, q] result gives [q, 65] where
    normalization (multiply by reciprocal of col 64) is a natural
    per-partition tensor_scalar op.
No max-subtraction in softmax: |scores| <~ 50 for randn-scale inputs,
exp fits fp32 comfortably (reference subtracts max; mathematically the
same ratio).

Matmuls run as float32r (4-byte storage, 1 PE cycle/row vs 4 for fp32,
tf32-like precision; measured output rel err ~8e-4 vs fp32 reference).
"""
import numpy as np

import concourse.bass as bass
import concourse.mybir as mybir
import concourse.tile as tile
from concourse.bass_utils import run_bass_kernel_spmd
from concourse.masks import make_identity

B, S, H = 2, 2048, 1024
NH, HD = 16, 64
N_CORES = 8
HPC = NH // (N_CORES // B)   # 4 heads per core
F = HPC * HD                 # 256 features per core
NEG = -10000000000.0

F32 = mybir.dt.float32
F32R = mybir.dt.float32r
MMDT = F32R


def _legalize_sync(nc, max_waits=1, max_updates=1):
    """This walrus build supports at most 1 sync wait / 1 sync update per
    instruction; split excess waits onto preceding same-engine NoOps."""
    n_upd = 0
    for f in nc.m.functions:
        for blk in f.blocks:
            out = []
            for inst in blk.instructions:
                si = getattr(inst, "sync_info", None)
                if si is not None and len(si.on_wait) > max_waits:
                    waits = list(si.on_wait)
                    for k, w in enumerate(waits[:-max_waits]):
                        out.append(mybir.InstNoOp(
                            name=f"{inst.name}-wsplit{k}",
                            sync_info=mybir.SyncInfo(on_wait=[w], on_update=[]),
                            bass_nofuse=True,
                            engine=inst.engine,
                        ))
                    inst.sync_info = mybir.SyncInfo(
                        on_wait=waits[-max_waits:], on_update=list(si.on_update))
                si = getattr(inst, "sync_info", None)
                if si is not None and len(si.on_update) > max_updates:
                    n_upd += 1
                out.append(inst)
            blk.instructions = out
    if n_upd:
        raise RuntimeError(f"{n_upd} instructions need >1 sync updates")


def _groups(total):
    """Split `total` positions into DMA/proj groups of <=512 (multiples
    of 256 so float32r matmuls stay at full rate)."""
    out = []
    pos = 0
    while pos < total:
        w = min(512, total - pos)
        out.append((pos, w))
        pos += w
    return out


def _emit(nc, tc, d, s_kv):
    from contextlib import ExitStack
    Exp = mybir.ActivationFunctionType.Exp
    Ident = mybir.ActivationFunctionType.Identity
    NQ = S // 512        # 4 query groups of 512
    NTQ = S // 128       # 16 query tiles of 128
    NTK = s_kv // 128    # key tiles of 128
    VW = F + HPC         # 260: V row-block width incl. ones columns

    with ExitStack() as ctx:
        const = ctx.enter_context(tc.tile_pool(name="const", bufs=1))
        ident32 = const.tile([128, 128], F32, tag="ident32", name="ident32")
        make_identity(nc, ident32)
        bqk_sb = const.tile([128, 4], F32, tag="bqk", name="bqk")
        nc.gpsimd.dma_start(bqk_sb[:, 0:2], d["bqr"])
        nc.gpsimd.dma_start(bqk_sb[:, 2:4], d["bkr"])
        mb_sb = const.tile([128, NTK], F32, tag="mb", name="mb")
        nc.gpsimd.dma_start(mb_sb[:], d["mbias"])
        bvb = const.tile([128, F], F32, tag="bvb", name="bvb")
        nc.gpsimd.dma_start(bvb[:], d["bvr"].to_broadcast((128, F)))

        qkv = ctx.enter_context(tc.tile_pool(name="qkv", bufs=1))
        # QT split per (m, 512-group) so attention on early q-groups can
        # start while later q-groups are still projecting
        QTt = [[qkv.tile([128, 512], MMDT, tag=f"qt{m}_{g}", name=f"qt{m}_{g}")
                for g in range(NQ)] for m in range(2)]
        KTm = [qkv.tile([128, s_kv], MMDT, tag=f"kt{m}", name=f"kt{m}")
               for m in range(2)]
        Vt = [qkv.tile([128, VW], MMDT, tag=f"v{t}", name=f"v{t}")
              for t in range(NTK)]
        outp = [qkv.tile([128, F], F32, tag=f"out{t}", name=f"out{t}")
                for t in range(NTQ)]
        for t in range(NTK):
            nc.gpsimd.memset(Vt[t][:].bitcast(mybir.dt.int32), 0x3F800000)

        wT_p = ctx.enter_context(tc.tile_pool(name="wT", bufs=1))
        xT_p = ctx.enter_context(tc.tile_pool(name="xT", bufs=3))
        es_p = ctx.enter_context(tc.tile_pool(name="expS", bufs=5))
        oT_p = ctx.enter_context(tc.tile_pool(name="oT", bufs=2))
        sm_p = ctx.enter_context(tc.tile_pool(name="sm", bufs=4))
        # 1-bank projection psum, alive through the whole kernel so the
        # q-projection overlaps ACT-bound attention (PSUM: 1+4+2+1 = 8)
        ps_qk = ctx.enter_context(
            tc.tile_pool(name="ps_qk", bufs=1, space="PSUM"))

        w_sb = {}
        for nm in ("wk", "wv", "wq"):
            w = wT_p.tile([128, 8 * F], MMDT, tag=nm, name=nm)
            nc.gpsimd.dma_start(
                w[:].rearrange("p (c f) -> p c f", c=8),
                d[nm + "T"].rearrange("(c p) f -> p c f", p=128))
            w_sb[nm] = w

        def load_xt(x_d, gpos, gw, slen):
            xT = xT_p.tile([128, 8 * 512], MMDT, tag="xT", name="xT")
            xTv = xT[:, 0:8 * gw].rearrange("p (c b) -> p c b", c=8)
            for c in range(8):
                nc.sync.dma_start(
                    xTv[:, c, :],
                    x_d[128 * c:128 * (c + 1), gpos:gpos + gw])
            return xTv

        # ---- K projection first (attention scores only need K) ----
        for gi, (gpos, gw) in enumerate(_groups(s_kv)):
            xTv = load_xt(d["xkT"], gpos, gw, s_kv)
            for m in range(2):
                pq = ps_qk.tile([128, 512], F32, tag="pq", name="pq")
                for c in range(8):
                    nc.tensor.matmul(
                        pq[:, 0:gw],
                        w_sb["wk"][:, 256 * c + 128 * m:
                                   256 * c + 128 * (m + 1)],
                        xTv[:, c, :],
                        start=(c == 0), stop=(c == 7))
                nc.vector.tensor_scalar(
                    KTm[m][:, gpos:gpos + gw], pq[:, 0:gw],
                    bqk_sb[:, 2 + m:3 + m], None, op0=mybir.AluOpType.add)


        def q_group(gi, gpos, gw):
            xTv = load_xt(d["xqT"], gpos, gw, S)
            for m in range(2):
                pq = ps_qk.tile([128, 512], F32, tag="pq", name="pq")
                for c in range(8):
                    nc.tensor.matmul(
                        pq[:],
                        w_sb["wq"][:, 256 * c + 128 * m:
                                   256 * c + 128 * (m + 1)],
                        xTv[:, c, :],
                        start=(c == 0), stop=(c == 7))
                nc.vector.tensor_scalar(
                    QTt[m][gi][:], pq[:], bqk_sb[:, m:m + 1], None,
                    op0=mybir.AluOpType.add)

        # Q group 0 right after K so attention can start; groups 1-3
        # after V, overlapping the ACT-bound attention phase.
        q_group(0, 0, 512)



        # ---- static PSUM plan: qk 1 + v 1 + scores 3 + acc 2 + tr 1 = 8
        # banks, all pools alive for the whole kernel so attention,
        # V-projection and q-projections interleave freely ----
        ps_vt = ctx.enter_context(
            tc.tile_pool(name="ps_vt", bufs=1, space="PSUM"))
        ps_s = ctx.enter_context(
            tc.tile_pool(name="ps_s", bufs=2, space="PSUM"))
        ps_o = ctx.enter_context(
            tc.tile_pool(name="ps_o", bufs=2, space="PSUM"))

        gv = _groups(s_kv)

        def v_group(vg):
            # V projection into per-kt tiles; attention PV matmuls chase
            # these tile by tile.
            gpos, gw = gv[vg]
            xTv = load_xt(d["xvT"], gpos, gw, s_kv)
            for j in range(gw // 128):
                pv = ps_vt.tile([128, F], F32, tag="pvt", name="pv")
                for c in range(8):
                    nc.tensor.matmul(
                        pv[:],
                        xTv[:, c, 128 * j:128 * (j + 1)],
                        w_sb["wv"][:, 256 * c:256 * (c + 1)],
                        start=(c == 0), stop=(c == 7))
                t = (gpos // 128) + j
                nc.vector.tensor_copy(
                    Vt[t][:].rearrange("p (h e) -> p h e", e=65)[:, :, 0:64],
                    pv[:].rearrange("p (h e) -> p h e", h=HPC))

        def c_group(g, v_after=()):
            for m in range(2):
                pv_lag = 2 if (v_after and m == 0) else 0
                # heads 2m (PE array rows 0-63) and 2m+1 (rows 64-127):
                # the two score matmuls go to different PE row groups
                # (tile_position auto-derived from base_partition) and
                # run concurrently in the array.
                acc0 = ps_o.tile([128, 512], F32, tag="acc", name="acc")
                acc1 = ps_o.tile([128, 512], F32, tag="acc", name="acc")
                h0, h1 = 2 * m, 2 * m + 1
                esq = []
                for kt in range(NTK + pv_lag):
                    # emit upcoming V projection groups just before their
                    # first PV consumer, so attention scores/exp start
                    # after only K+Q0 have loaded
                    if m == 0:
                        for vg, vstart in v_after:
                            if kt == vstart:
                                v_group(vg)
                    if kt < NTK:
                        ksl = slice(128 * kt, 128 * (kt + 1))
                        ps = ps_s.tile([128, 1024], F32, tag="ps", name="ps")
                        nc.tensor.matmul(
                            ps[:, 0:512], KTm[m][0:64, ksl],
                            QTt[m][g][0:64, :],
                            start=True, stop=True)
                        nc.tensor.matmul(
                            ps[:, 512:1024], KTm[m][64:128, ksl],
                            QTt[m][g][64:128, :],
                            start=True, stop=True)
                        es = es_p.tile([128, 1024], MMDT, tag="es", name="es")
                        nc.scalar.activation(
                            es[:], ps[:], Exp, bias=mb_sb[:, kt:kt + 1])
                        esq.append(es)
                    if kt >= pv_lag:
                        pk = kt - pv_lag
                        nc.tensor.matmul(
                            acc0[0:65, :], Vt[pk][:, 65 * h0:65 * (h0 + 1)],
                            esq[pk][:, 0:512],
                            start=(pk == 0), stop=(pk == NTK - 1))
                        nc.tensor.matmul(
                            acc1[0:65, :], Vt[pk][:, 65 * h1:65 * (h1 + 1)],
                            esq[pk][:, 512:1024],
                            start=(pk == 0), stop=(pk == NTK - 1))
                oT = oT_p.tile([128, 1024], F32, tag="oT", name="oT")
                nc.vector.tensor_copy(oT[0:65, 0:512], acc0[0:65, :])
                nc.vector.tensor_copy(oT[0:65, 512:1024], acc1[0:65, :])
                for hh in range(2):
                    h = 2 * m + hh
                    for j in range(4):
                        pt = ps_vt.tile([128, 65], F32, tag="pvt", name="ptt")
                        nc.tensor.transpose(
                            pt[:, 0:65],
                            oT[0:65, 512 * hh + 128 * j:512 * hh + 128 * (j + 1)],
                            ident32[0:65, 0:65])
                        rc = sm_p.tile([128, 1], F32, tag="rc", name="rc")
                        nc.vector.reciprocal(rc[:], pt[:, 64:65])
                        tmp = sm_p.tile([128, 64], F32, tag="tmp", name="tmp")
                        nc.vector.tensor_scalar_mul(tmp[:], pt[:, 0:64], rc[:])
                        nc.vector.tensor_add(
                            outp[4 * g + j][:, 64 * h:64 * (h + 1)],
                            tmp[:], bvb[:, 64 * h:64 * (h + 1)])
            for j in range(4):
                nc.sync.dma_start(
                    d["out"][512 * g + 128 * j:512 * g + 128 * (j + 1), :],
                    outp[4 * g + j][:])

        # attention group 0 first (needs only K + Q0 + the first V
        # group); later V groups are emitted inside C0's kt loop just
        # ahead of their consumers, and the other q-projections fill PE
        # gaps under the ACT-bound attention.
        gq = _groups(S)
        c_group(0, v_after=[(0, 2), (1, 4), (2, 7)][:len(gv)])
        for g in range(1, NQ):
            q_group(g, *gq[g])
            c_group(g)


_NC_CACHE = {}


def _build(s_kv):
    if s_kv in _NC_CACHE:
        return _NC_CACHE[s_kv]
    nc = bass.Bass(trn_type="TRN2", target_bir_lowering=False, debug=False)
    d = {
        "xqT": nc.dram_tensor("xqT", [H, S], MMDT, kind="ExternalInput").ap(),
        "xkT": nc.dram_tensor("xkT", [H, s_kv], MMDT, kind="ExternalInput").ap(),
        "xvT": nc.dram_tensor("xvT", [H, s_kv], MMDT, kind="ExternalInput").ap(),
        "wqT": nc.dram_tensor("wqT", [H, F], MMDT, kind="ExternalInput").ap(),
        "wkT": nc.dram_tensor("wkT", [H, F], MMDT, kind="ExternalInput").ap(),
        "wvT": nc.dram_tensor("wvT", [H, F], MMDT, kind="ExternalInput").ap(),
        "bqr": nc.dram_tensor("bqr", [128, 2], F32, kind="ExternalInput").ap(),
        "bkr": nc.dram_tensor("bkr", [128, 2], F32, kind="ExternalInput").ap(),
        "bvr": nc.dram_tensor("bvr", [1, F], F32, kind="ExternalInput").ap(),
        "mbias": nc.dram_tensor("mbias", [128, s_kv // 128], F32,
                                kind="ExternalInput").ap(),
        "out": nc.dram_tensor("out", [S, F], F32, kind="ExternalOutput").ap(),
    }
    with tile.TileContext(nc) as tc:
        _emit(nc, tc, d, s_kv)
    _legalize_sync(nc)
    _NC_CACHE[s_kv] = nc
    return nc


def plan_kv(mask):
    """Per-batch compaction plan: indices of valid key positions and the
    padded kv length shared across batches."""
    mask = np.asarray(mask)
    idxs = [np.nonzero(mask[b])[0] for b in range(B)]
    nmax = max((len(i) for i in idxs), default=1)
    s_kv = min(S, max(256, -(-nmax // 256) * 256))
    return idxs, s_kv


def make_in_maps(query, key, value, mask, Wq, bq, Wk, bk, Wv, bv,
                 idxs=None, s_kv=None):
    if idxs is None:
        idxs, s_kv = plan_kv(mask)
    query, key, value = (np.asarray(a, np.float32) for a in (query, key, value))
    Wq, Wk, Wv = (np.asarray(a, np.float32) for a in (Wq, Wk, Wv))
    bq, bk, bv = (np.asarray(a, np.float32) for a in (bq, bk, bv))
    in_maps = []
    qc, kc, vc, mbc = {}, {}, {}, {}
    for b in range(B):
        idx = idxs[b]
        qc[b] = np.ascontiguousarray(query[b].T)
        kcb = np.zeros((H, s_kv), np.float32)
        kcb[:, :len(idx)] = key[b][idx].T
        vcb = np.zeros((H, s_kv), np.float32)
        vcb[:, :len(idx)] = value[b][idx].T
        mb = np.full(s_kv, NEG, np.float32)
        mb[:len(idx)] = 0.0
        kc[b], vc[b] = kcb, vcb
        mbc[b] = np.ascontiguousarray(mb.reshape(s_kv // 128, 128).T)
    for c in range(N_CORES):
        b = c // (N_CORES // B)
        fs = F * (c % (N_CORES // B))
        in_maps.append({
            "xqT": qc[b],
            "xkT": kc[b],
            "xvT": vc[b],
            "wqT": np.ascontiguousarray(Wq[fs:fs + F].T),
            "wkT": np.ascontiguousarray(Wk[fs:fs + F].T),
            "wvT": np.ascontiguousarray(Wv[fs:fs + F].T),
            "bqr": np.ascontiguousarray(bq[fs:fs + F].reshape(2, 128).T),
            "bkr": np.ascontiguousarray(bk[fs:fs + F].reshape(2, 128).T),
            "bvr": np.ascontiguousarray(bv[fs:fs + F].reshape(1, F)),
            "mbias": mbc[b],
        })
    return in_maps


def assemble(results):
    out = np.empty((B, S, H), np.float32)
    for c in range(N_CORES):
        b = c // (N_CORES // B)
        fs = F * (c % (N_CORES // B))
        out[b, :, fs:fs + F] = results[c]["out"]
    return out


def kernel(query, key, value, mask, Wq, bq, Wk, bk, Wv, bv, _trace=False):
    idxs, s_kv = plan_kv(mask)
    nc = _build(s_kv)
    in_maps = make_in_maps(query, key, value, mask, Wq, bq, Wk, bk, Wv, bv,
                           idxs, s_kv)
    res = run_bass_kernel_spmd(nc, in_maps, core_ids=list(range(N_CORES)),
                               trace=_trace)
    out = assemble(res.results)
    if _trace:
        return out, res
    return out



# revision 3
# speedup vs baseline: 1.1570x; 1.1570x over previous
"""Multi-head attention Bass kernel for Trainium2, 8-core SPMD.

Problem: B=2, S=2048, H=1024, 16 heads of 64 (torch-style MHA without
1/sqrt(d) scaling, key-padding mask, eval mode).

Sharding: core c handles batch b = c//4 and 4 heads (feature slice
256*(c%4) .. +256). Each core computes Q/K/V projections for its feature
slice over its batch, then attention for its 4 heads, producing
out[b, :, fslice]. Host concatenates.

Key-padding compaction: masked key positions contribute exactly 0 to the
softmax numerator and denominator, so the host drops masked key/value
rows and pads to a multiple of 128. Padded key columns are zero, so
their scores are 0 and exp gives 1; those rows contribute nothing
because the V rows are zero AND the per-head "ones" column of V (which
accumulates the softmax denominator through the PV matmul) is zeroed at
padded rows via a host-provided validity column.

Device-side layout (everything "transposed", features on partitions):
  - projections: W^T (stationary, bf16) x X^T (moving, bf16) -> psum;
    K^T/Q^T stored fp32r, V stored bf16 as [kpos, 4*(64+1)] with the
    +1 validity column per head
  - scores S^T[kpos, q] = K^T.T @ Q^T per head (contraction d=64),
    512 q at a time, two heads per psum tile
  - es = exp(scores) -> bf16 SBUF
  - PV runs FLIPPED: out[q, 65] += es_slice[k,128q].T @ V[k, 65] --
    the 65-column moving operand costs 65 rows/matmul instead of 512,
    and the output lands [q, d]-oriented so no PE transpose is needed;
    column 64 is the softmax denominator
  - normalization: reciprocal of col 64, then (out*rc)+bv via
    scalar_tensor_tensor, directly into the [q, F] output tile

Schedule: 8 phases (4 q-groups x 2 head-pairs). Phase p emits its
scores+exp stream (Activation-paced), the PV matmuls of phase p-1, and
"filler" projection chunks (V-proj per kv tile, Q-proj for later
q-groups) sized to hide in the exp-paced slots. A PE warmup (5 dummy
matmuls at t~0) ramps the tensor engine to full p-state before real
work arrives.
"""
import numpy as np
import ml_dtypes

import concourse.bass as bass
import concourse.mybir as mybir
import concourse.tile as tile
from concourse.bass_utils import run_bass_kernel_spmd

B, S, H = 2, 2048, 1024
NH, HD = 16, 64
N_CORES = 8
HPC = NH // (N_CORES // B)   # 4 heads per core
F = HPC * HD                 # 256 features per core

F32 = mybir.dt.float32
F32R = mybir.dt.float32r
BF16 = mybir.dt.bfloat16
NPBF16 = ml_dtypes.bfloat16


def _legalize_sync(nc, max_waits=1, max_updates=1):
    """This walrus build supports at most 1 sync wait / 1 sync update per
    instruction; split excess waits onto preceding same-engine NoOps."""
    n_upd = 0
    for f in nc.m.functions:
        for blk in f.blocks:
            out = []
            for inst in blk.instructions:
                si = getattr(inst, "sync_info", None)
                if si is not None and len(si.on_wait) > max_waits:
                    waits = list(si.on_wait)
                    for k, w in enumerate(waits[:-max_waits]):
                        out.append(mybir.InstNoOp(
                            name=f"{inst.name}-wsplit{k}",
                            sync_info=mybir.SyncInfo(on_wait=[w], on_update=[]),
                            bass_nofuse=True,
                            engine=inst.engine,
                        ))
                    inst.sync_info = mybir.SyncInfo(
                        on_wait=waits[-max_waits:], on_update=list(si.on_update))
                si = getattr(inst, "sync_info", None)
                if si is not None and len(si.on_update) > max_updates:
                    n_upd += 1
                out.append(inst)
            blk.instructions = out
    if n_upd:
        raise RuntimeError(f"{n_upd} instructions need >1 sync updates")


def _groups(total):
    """Split positions into DMA/proj groups of <=512 (multiples of 128)."""
    out = []
    pos = 0
    while pos < total:
        w = min(512, total - pos)
        out.append((pos, w))
        pos += w
    return out


def _emit(nc, tc, d, s_kv):
    from contextlib import ExitStack
    from collections import deque
    Exp = mybir.ActivationFunctionType.Exp
    add = mybir.AluOpType.add
    mult = mybir.AluOpType.mult
    NQ = S // 512          # 4 query groups of 512
    NTQ = S // 128         # 16 query tiles of 128
    NTK = s_kv // 128      # kv tiles of 128
    VW = HPC * (HD + 1)    # 260: V row-block width incl. validity columns
    kgroups = _groups(s_kv)
    qgroups = _groups(S)

    with ExitStack() as ctx:
        const = ctx.enter_context(tc.tile_pool(name="const", bufs=1))
        bqk_sb = const.tile([128, 4], F32, tag="bqk", name="bqk")
        nc.gpsimd.dma_start(bqk_sb[:, 0:2], d["bqr"])
        nc.gpsimd.dma_start(bqk_sb[:, 2:4], d["bkr"])
        bvb = const.tile([128, F], F32, tag="bvb", name="bvb")
        nc.gpsimd.dma_start(bvb[:], d["bvr"].to_broadcast((128, F)))
        vone = const.tile([128, NTK], BF16, tag="vone", name="vone")
        nc.gpsimd.dma_start(vone[:], d["vones"])
        warm = const.tile([128, 640], BF16, tag="warm", name="warm")
        nc.gpsimd.memset(warm[:].bitcast(mybir.dt.int32), 0)

        qkv = ctx.enter_context(tc.tile_pool(name="qkv", bufs=1))
        KT = [qkv.tile([128, s_kv], F32R, tag=f"kt{m}", name=f"kt{m}")
              for m in range(2)]
        QT = [qkv.tile([128, S], F32R, tag=f"qt{m}", name=f"qt{m}")
              for m in range(2)]
        Vt = [qkv.tile([128, VW], BF16, tag=f"v{t}", name=f"v{t}")
              for t in range(NTK)]
        outp = [qkv.tile([128, F], F32, tag=f"out{t}", name=f"out{t}")
                for t in range(NTQ)]
        # validity column per head (zeroes the denominator contribution of
        # padded key rows); V data columns are fully written by V-proj
        for t in range(NTK):
            for h in range(HPC):
                nc.gpsimd.tensor_copy(
                    Vt[t][:, 65 * h + 64:65 * h + 65], vone[:, t:t + 1])

        wT_p = ctx.enter_context(tc.tile_pool(name="wT", bufs=1))
        xT_p = ctx.enter_context(tc.tile_pool(name="xT", bufs=5))
        es_p = ctx.enter_context(tc.tile_pool(name="expS", bufs=14))
        sm_p = ctx.enter_context(tc.tile_pool(name="sm", bufs=2))
        rc_p = ctx.enter_context(tc.tile_pool(name="rc", bufs=8))

        # PSUM: proj 1 bank + scores 2x2 banks + PV accumulators
        pj_p = ctx.enter_context(tc.tile_pool(name="pj", bufs=1, space="PSUM"))
        ps_p = ctx.enter_context(tc.tile_pool(name="ps", bufs=2, space="PSUM"))
        pa_p = ctx.enter_context(tc.tile_pool(name="pa", bufs=1, space="PSUM"))

        # ---- PE warmup: ramp the tensor engine p-state before real work ----
        for _ in range(5):
            pw = ps_p.tile([128, 1024], F32, tag="ps", name="warmmm")
            nc.tensor.matmul(pw[:, 0:512], warm[:, 0:128], warm[:, 128:640],
                             start=True, stop=True)

        def load_w(nm):
            w = wT_p.tile([128, 8 * F], BF16, tag=nm, name=nm)
            nc.sync.dma_start(
                w[:].rearrange("p (c f) -> p c f", c=8),
                d[nm + "T"].rearrange("(c p) f -> p c f", p=128))
            return w

        def load_xt(x_d, gpos, gw):
            xT = xT_p.tile([128, 8 * 512], BF16, tag="xT", name="xT")
            xTv = xT[:, 0:8 * gw].rearrange("p (c b) -> p c b", c=8)
            for c in range(8):
                nc.sync.dma_start(
                    xTv[:, c, :],
                    x_d[128 * c:128 * (c + 1), gpos:gpos + gw])
            return xTv

        w_sb = {}
        # K first (scores need K); interleave W loads with K x-loads so the
        # first K-proj group starts as early as possible
        w_sb["wk"] = load_w("wk")
        xt_k = []
        for gi, (gpos, gw) in enumerate(kgroups):
            if gi == 1:
                w_sb["wq"] = load_w("wq")
            if gi == 2:
                w_sb["wv"] = load_w("wv")
            xt_k.append(load_xt(d["xkT"], gpos, gw))
        if "wq" not in w_sb:
            w_sb["wq"] = load_w("wq")
        if "wv" not in w_sb:
            w_sb["wv"] = load_w("wv")

        def kq_proj(w, xTv, gw, dst, m, bias_col):
            pq = pj_p.tile([128, 512], F32, tag="pj", name="pj")
            for c in range(8):
                nc.tensor.matmul(
                    pq[:, 0:gw],
                    w[:, 256 * c + 128 * m:256 * c + 128 * (m + 1)],
                    xTv[:, c, 0:gw],
                    start=(c == 0), stop=(c == 7))
            nc.vector.tensor_scalar(
                dst, pq[:, 0:gw], bias_col, None, op0=add)

        # K projection (all of it -- scores for any kt need it early)
        for gi, (gpos, gw) in enumerate(kgroups):
            for m in range(2):
                kq_proj(w_sb["wk"], xt_k[gi], gw,
                        KT[m][:, gpos:gpos + gw], m, bqk_sb[:, 2 + m:3 + m])

        # Q projection group 0
        xt_q0 = load_xt(d["xqT"], 0, 512)
        for m in range(2):
            kq_proj(w_sb["wq"], xt_q0, 512,
                    QT[m][:, 0:512], m, bqk_sb[:, m:m + 1])

        # V x-loads up front (DMA queue order; consumed by V-proj fillers)
        xt_v = [load_xt(d["xvT"], gpos, gw) for gpos, gw in kgroups]

        # ---- filler chunks (small PE units scheduled into exp-paced slots)
        fillers = deque()
        v_psum = {}

        def v_chunk(t, half):
            # V-proj for kv tile t: 8 accumulation matmuls in 2 chunks
            gi = next(i for i, (gp, gw) in enumerate(kgroups)
                      if gp <= 128 * t < gp + gw)
            j = t - kgroups[gi][0] // 128
            def emit():
                if half == 0:
                    v_psum[t] = pj_p.tile([128, 512], F32, tag="pj", name="pvt")
                pv = v_psum[t]
                for c in (range(4) if half == 0 else range(4, 8)):
                    nc.tensor.matmul(
                        pv[:, 0:F],
                        xt_v[gi][:, c, 128 * j:128 * (j + 1)],
                        w_sb["wv"][:, 256 * c:256 * (c + 1)],
                        start=(c == 0), stop=(c == 7))
                if half == 1:
                    nc.vector.tensor_copy(
                        Vt[t][:].rearrange(
                            "p (h e) -> p h e", e=HD + 1)[:, :, 0:HD],
                        pv[:, 0:F].rearrange("p (h e) -> p h e", h=HPC))
            return emit

        def q_chunk(g, m, part):
            # Q-proj group g, head-pair m: 8 matmuls in 4 chunks of 2
            gpos, gw = qgroups[g]
            def emit():
                if part == 0 and m == 0:
                    # x-load for this q-group rides with the first chunk
                    xt_q[g] = load_xt(d["xqT"], gpos, gw)
                if part == 0:
                    q_psum[(g, m)] = pj_p.tile([128, 512], F32, tag="pj",
                                               name="pjq")
                pq = q_psum[(g, m)]
                for c in range(2 * part, 2 * part + 2):
                    nc.tensor.matmul(
                        pq[:],
                        w_sb["wq"][:, 256 * c + 128 * m:256 * c + 128 * (m + 1)],
                        xt_q[g][:, c, :],
                        start=(c == 0), stop=(c == 7))
                if part == 3:
                    nc.vector.tensor_scalar(
                        QT[m][:, gpos:gpos + gw], pq[:],
                        bqk_sb[:, m:m + 1], None, op0=add)
            return emit

        xt_q = {0: xt_q0}
        q_psum = {}
        for m in range(2):
            for part in range(4):
                fillers.append(q_chunk(1, m, part))
        for t in range(NTK):
            for half in range(2):
                fillers.append(v_chunk(t, half))
        for g in (2, 3):
            for m in range(2):
                for part in range(4):
                    fillers.append(q_chunk(g, m, part))

        def pop_fillers(n):
            for _ in range(n):
                if fillers:
                    fillers.popleft()()

        # ---- attention phases ----
        def scores_exp(g, m, kt):
            ksl = slice(128 * kt, 128 * (kt + 1))
            qs = slice(512 * g, 512 * (g + 1))
            ps_t = ps_p.tile([128, 1024], F32, tag="ps", name="ps")
            nc.tensor.matmul(ps_t[:, 0:512], KT[m][0:64, ksl],
                             QT[m][0:64, qs], start=True, stop=True)
            nc.tensor.matmul(ps_t[:, 512:1024], KT[m][64:128, ksl],
                             QT[m][64:128, qs], start=True, stop=True)
            es_t = es_p.tile([128, 1024], BF16, tag="es", name="es")
            nc.scalar.activation(es_t[:], ps_t[:], Exp)
            return es_t

        def pv_emit(state, kt):
            g, m, es_list, acc = state
            for h in range(2):
                hh = 2 * m + h
                for j in range(4):
                    a = 4 * h + j
                    nc.tensor.matmul(
                        acc[:, 65 * a:65 * (a + 1)],
                        es_list[kt][:, 512 * h + 128 * j:512 * h + 128 * (j + 1)],
                        Vt[kt][:, 65 * hh:65 * (hh + 1)],
                        start=(kt == 0), stop=(kt == NTK - 1))

        def finish(state):
            g, m, es_list, acc = state
            smt = sm_p.tile([128, 520], F32, tag="sm", name="sm")
            nc.vector.tensor_copy(smt[:], acc[:])
            for h in range(2):
                hh = 2 * m + h
                for j in range(4):
                    a = 4 * h + j
                    rc = rc_p.tile([128, 1], F32, tag="rc", name="rc")
                    nc.vector.reciprocal(rc[:], smt[:, 65 * a + 64:65 * a + 65])
                    nc.vector.scalar_tensor_tensor(
                        outp[4 * g + j][:, 64 * hh:64 * (hh + 1)],
                        smt[:, 65 * a:65 * a + 64], rc[:],
                        bvb[:, 64 * hh:64 * (hh + 1)],
                        op0=mult, op1=add)
            if m == 1:
                for j in range(4):
                    qt = 4 * g + j
                    nc.sync.dma_start(
                        d["out"][128 * qt:128 * (qt + 1), :], outp[qt][:])

        prev = None
        for p, (g, m) in enumerate((g, m) for g in range(NQ) for m in range(2)):
            es_list = []
            acc = pa_p.tile([128, 520], F32, tag="acc", name="acc")
            st = (g, m, es_list, acc)
            for kt in range(NTK):
                es_list.append(scores_exp(g, m, kt))
                if p == 0:
                    # Q-proj g1 chunks late in phase 0 (x lands mid-phase)
                    if kt >= NTK - 4:
                        pop_fillers(2)
                elif p == 1:
                    # V-proj chunks, 1-slot lead over their PV consumers
                    pop_fillers(4 if kt == 0 else 2)
                    pv_emit(prev, kt)
                else:
                    pop_fillers(1)
                    pv_emit(prev, kt)
            if prev is not None:
                finish(prev)
            prev = st
        pop_fillers(len(fillers))
        for kt in range(NTK):
            pv_emit(prev, kt)
        finish(prev)


_NC_CACHE = {}


def _build(s_kv):
    if s_kv in _NC_CACHE:
        return _NC_CACHE[s_kv]
    nc = bass.Bass(trn_type="TRN2", target_bir_lowering=False, debug=False)
    NTK = s_kv // 128
    d = {
        "xqT": nc.dram_tensor("xqT", [H, S], BF16, kind="ExternalInput").ap(),
        "xkT": nc.dram_tensor("xkT", [H, s_kv], BF16, kind="ExternalInput").ap(),
        "xvT": nc.dram_tensor("xvT", [H, s_kv], BF16, kind="ExternalInput").ap(),
        "wqT": nc.dram_tensor("wqT", [H, F], BF16, kind="ExternalInput").ap(),
        "wkT": nc.dram_tensor("wkT", [H, F], BF16, kind="ExternalInput").ap(),
        "wvT": nc.dram_tensor("wvT", [H, F], BF16, kind="ExternalInput").ap(),
        "bqr": nc.dram_tensor("bqr", [128, 2], F32, kind="ExternalInput").ap(),
        "bkr": nc.dram_tensor("bkr", [128, 2], F32, kind="ExternalInput").ap(),
        "bvr": nc.dram_tensor("bvr", [1, F], F32, kind="ExternalInput").ap(),
        "vones": nc.dram_tensor("vones", [128, NTK], BF16,
                                kind="ExternalInput").ap(),
        "out": nc.dram_tensor("out", [S, F], F32, kind="ExternalOutput").ap(),
    }
    with tile.TileContext(nc) as tc:
        _emit(nc, tc, d, s_kv)
    _legalize_sync(nc)
    _NC_CACHE[s_kv] = nc
    return nc


def plan_kv(mask):
    """Per-batch compaction plan: indices of valid key positions and the
    padded kv length (multiple of 128) shared across batches."""
    mask = np.asarray(mask)
    idxs = [np.nonzero(mask[b])[0] for b in range(B)]
    nmax = max((len(i) for i in idxs), default=1)
    s_kv = min(S, max(128, -(-nmax // 128) * 128))
    return idxs, s_kv


def make_in_maps(query, key, value, mask, Wq, bq, Wk, bk, Wv, bv,
                 idxs=None, s_kv=None):
    if idxs is None:
        idxs, s_kv = plan_kv(mask)
    NTK = s_kv // 128
    query, key, value = (np.asarray(a, np.float32) for a in (query, key, value))
    Wq, Wk, Wv = (np.asarray(a, np.float32) for a in (Wq, Wk, Wv))
    bq, bk, bv = (np.asarray(a, np.float32) for a in (bq, bk, bv))
    in_maps = []
    qc, kc, vc, vo = {}, {}, {}, {}
    for b in range(B):
        idx = idxs[b]
        qc[b] = np.ascontiguousarray(query[b].T.astype(NPBF16))
        kcb = np.zeros((H, s_kv), NPBF16)
        kcb[:, :len(idx)] = key[b][idx].T.astype(NPBF16)
        vcb = np.zeros((H, s_kv), NPBF16)
        vcb[:, :len(idx)] = value[b][idx].T.astype(NPBF16)
        kc[b], vc[b] = kcb, vcb
        v1 = np.zeros(s_kv, NPBF16)
        v1[:len(idx)] = 1.0
        vo[b] = np.ascontiguousarray(v1.reshape(NTK, 128).T)
    for c in range(N_CORES):
        b = c // (N_CORES // B)
        fs = F * (c % (N_CORES // B))
        in_maps.append({
            "xqT": qc[b],
            "xkT": kc[b],
            "xvT": vc[b],
            "wqT": np.ascontiguousarray(Wq[fs:fs + F].T.astype(NPBF16)),
            "wkT": np.ascontiguousarray(Wk[fs:fs + F].T.astype(NPBF16)),
            "wvT": np.ascontiguousarray(Wv[fs:fs + F].T.astype(NPBF16)),
            "bqr": np.ascontiguousarray(bq[fs:fs + F].reshape(2, 128).T),
            "bkr": np.ascontiguousarray(bk[fs:fs + F].reshape(2, 128).T),
            "bvr": np.ascontiguousarray(bv[fs:fs + F].reshape(1, F)),
            "vones": vo[b],
        })
    return in_maps


def assemble(results):
    out = np.empty((B, S, H), np.float32)
    for c in range(N_CORES):
        b = c // (N_CORES // B)
        fs = F * (c % (N_CORES // B))
        out[b, :, fs:fs + F] = results[c]["out"]
    return out


def kernel(query, key, value, mask, Wq, bq, Wk, bk, Wv, bv, _trace=False):
    idxs, s_kv = plan_kv(mask)
    nc = _build(s_kv)
    in_maps = make_in_maps(query, key, value, mask, Wq, bq, Wk, bk, Wv, bv,
                           idxs, s_kv)
    res = run_bass_kernel_spmd(nc, in_maps, core_ids=list(range(N_CORES)),
                               trace=_trace)
    out = assemble(res.results)
    if _trace:
        return out, res
    return out
